# revision 21
# baseline (speedup 1.0000x reference)
"""Trainium2 Bass kernel for BaseLinearLayerWithLoRA (moe_routing).

out = x @ W^T + b  +  per-token LoRA:  out[t] += (x[t] @ A[l]^T) @ B[l]^T,  l = idx[t]

Sharding: tokens are stably sorted by adapter id on the host and split into
8 contiguous shards of 4096; shard c goes to core c.  After sorting, each
shard is dominated by ONE adapter (the few boundary "minority" tokens of a
neighboring adapter are rotated to the shard's tail).  Each core therefore
runs a plain dense GEMM against a per-core FOLDED weight table
W_c = W + B_maj A_maj — the LoRA contribution of the majority adapter costs
zero device work.  Only the last super-block (which holds the <=512 minority
tokens) runs the stacked shrink/expand fixup, with a SIGNED mask (+1 on the
token's true adapter rows, -1 on the folded majority rows).  The host
scatters per-core outputs back through the sort permutation.

GEMM precision/throughput: dual-fp8 residual decomposition on the DoubleRow
perf mode (2 fp8 MACs per PE cell per cycle).  Host splits x = xa + xb and
s*W = wa + wb (xa/wa the e4m3 quantization, xb/wb the quantized residual,
s ~ 1/std(W) keeps wb clear of the fp8 denormal floor).  Three DoubleRow
passes per 256-deep chunk-pair — xa@wa, xa@wb, xb@wa — cost 0.75x the
cycles of one bf16 GEMM while the dropped xb@wb term is O(2^-8): measured
~1.3e-3 rms vs the fp32 reference, BETTER than the 2.3e-3 of bf16.
The 1/s descale rides the PSUM->SBUF bias-add drain (one DVE
scalar_tensor_tensor).

Schedule (per core, mirrors the bf16 v3 kernel):
  - x (both fp8 halves) host-retiled to [super-block, partition, chunk,
    token] so each super-block is one line-rate DMA per half.
  - Per block: 8 chunk-pairs x {stationary xa: 8 MMs (wa,wb o-sweep);
    stationary xb: 4 MMs (wa)} into 4 PSUM banks.
  - SB0 runs its blocks in PAIRS c-major (two blocks share each arriving W
    chunk-pair) so compute keeps pace with the serial DMA bus during the
    weight prologue; the small tables queue behind W on the SP ring.
  - The fixup shrink is 8 DoubleRow MMs (at8 = s*A stacked, fp8) interleaved
    1-per-chunk-pair into SB6 block 1; the expand is a single bf16 matmul
    per (block, o-tile) appended to the accumulation group.
  - Final block runs o-major so drains+stores overlap its own matmuls; the
    very last o-tile drains+stores in 256-col halves on alternating rings.
"""

import contextlib
import sys

for _p in ("/opt/trn_rl_repo", "/root/.axon_site/_ro/trn_rl_repo"):
    if _p not in sys.path:
        sys.path.insert(0, _p)

import numpy as np
import ml_dtypes

import concourse.bass as bass  # noqa: F401  (registers engines)
import concourse.mybir as mybir
import concourse.tile as tile
from concourse import bacc
from concourse.bass_utils import run_bass_kernel_spmd

N_CORES = 8
T_FULL, D_IN, D_OUT = 32768, 2048, 2048
MAX_LORAS, RANK = 8, 16
T_CORE = T_FULL // N_CORES          # 4096 tokens per core
SB_T = 512                          # super-block tokens
N_SB = T_CORE // SB_T               # 8 super-blocks
N_BLK = SB_T // 128                 # 4 token blocks per super-block
KC = D_IN // 128                    # 16 contraction chunks
CP = KC // 2                        # 8 DoubleRow chunk-pairs (256-deep)
N_OT = D_OUT // 512                 # 4 o-tiles (full width resident)

_CACHED = {}
_DR = mybir.MatmulPerfMode.DoubleRow


def _build(inv_s, reps=1, n_fix=1, lora=True, store=True, xdma=True):
    # reps>1 wraps the whole body in a device-side For_i loop — used only by
    # the timing harness to amortize launch overhead.  lora/store/xdma are
    # ablation switches (timing harness only — they break correctness).
    # n_fix = number of trailing super-blocks that run the minority fixup.
    # inv_s = compile-time descale immediate (1/s) for the drain.
    key = ("nc", float(inv_s), reps, n_fix, lora, store, xdma)
    if key in _CACHED:
        return _CACHED[key]
    fix_set = set(range(N_SB - n_fix, N_SB)) if lora else set()

    f32 = mybir.dt.float32
    bf16 = mybir.dt.bfloat16
    f8 = mybir.dt.float8e4

    nc = bacc.Bacc("TRN2", target_bir_lowering=False, debug=False)

    # x?P[s, p, c*SB_T + t] = x?[s*SB_T + t, c*128 + p]: one contiguous run
    # per (super-block, partition) -> line-rate DMA.  Chunk c = 2*cp + j maps
    # contraction element d = c*128 + p to DoubleRow pair slot j of pair cp.
    xaP = nc.dram_tensor("xaP", [N_SB * 128, KC * SB_T], f8, kind="ExternalInput")
    xbP = nc.dram_tensor("xbP", [N_SB * 128, KC * SB_T], f8, kind="ExternalInput")
    waT = nc.dram_tensor("waT", [D_IN, D_OUT], f8, kind="ExternalInput")
    wbT = nc.dram_tensor("wbT", [D_IN, D_OUT], f8, kind="ExternalInput")
    # aT8[p, c*128 + r] = s*A_stacked[r, c*128 + p]: partition-major so the
    # whole table is one flat DMA into the [128, CP, 2, 128] tile.
    aT8 = nc.dram_tensor("aT8", [128, KC * 128], f8, kind="ExternalInput")
    bT = nc.dram_tensor("bT", [128, D_OUT], bf16, kind="ExternalInput")
    maskM = nc.dram_tensor("maskM", [128, max(n_fix, 1) * SB_T], bf16,
                           kind="ExternalInput")
    bias_rep = nc.dram_tensor("bias_rep", [128, D_OUT], bf16, kind="ExternalInput")
    out = nc.dram_tensor("out", [T_CORE, D_OUT], f32, kind="ExternalOutput")

    xaP_v = xaP.rearrange("(s p) q -> p s q", p=128)    # [128, N_SB, KC*SB_T]
    xbP_v = xbP.rearrange("(s p) q -> p s q", p=128)
    waT_v = waT.rearrange("(c p) o -> p c o", p=128)    # [128, 16, 2048]
    wbT_v = wbT.rearrange("(c p) o -> p c o", p=128)

    with tile.TileContext(nc) as tc:
        with (
            tc.tile_pool(name="const", bufs=1) as const,
            tc.tile_pool(name="wpool", bufs=1) as wpool,
            tc.tile_pool(name="xpool", bufs=3) as xpool,
            tc.tile_pool(name="spool", bufs=2) as spool,
            tc.tile_pool(name="opool", bufs=3) as opool,
            tc.tile_pool(name="pso", bufs=8, space="PSUM") as pso,
        ):
            at8 = const.tile([128, CP, 2, 128], f8)
            bt = const.tile([128, D_OUT], bf16)
            bias_t = const.tile([128, D_OUT], bf16)
            mall = const.tile([128, max(n_fix, 1) * SB_T], bf16)
            wa = wpool.tile([128, CP, 2, D_OUT], f8)
            wb = wpool.tile([128, CP, 2, D_OUT], f8)

            rep_cm = tc.For_i(0, reps) if reps > 1 else contextlib.nullcontext()
            with rep_cm:
                def load_x(s, splits=(KC,), eng=None):
                    # loads BOTH fp8 halves; multiple split groups let the
                    # first block's matmuls start after the first group lands.
                    eng = eng or nc.sync
                    xa = xpool.tile([128, CP, 2, SB_T], f8, tag="xa",
                                    name=f"xa{s}")
                    xb = xpool.tile([128, CP, 2, SB_T], f8, tag="xb",
                                    name=f"xb{s}")
                    for xt, xv in ((xa, xaP_v), (xb, xbP_v)):
                        g0 = 0
                        for gs in splits:
                            eng.dma_start(
                                xt[:].rearrange("p c j t -> p (c j t)")
                                [:, g0 * SB_T:(g0 + gs) * SB_T],
                                xv[:, s, g0 * SB_T:(g0 + gs) * SB_T])
                            g0 += gs
                        assert g0 == KC
                    return xa, xb

                xts = [None] * N_SB
                # Prologue: the DMA bus is the serial resource; W pair-chunks
                # (wa+wb interleaved, both rings) must beat everything not
                # needed until later.  SB0's x rides SWDGE; small tables
                # queue on SP BEHIND the W chunks.
                xts[0] = load_x(0, splits=(2, 2, 4, 8), eng=nc.gpsimd)
                for cp in range(CP):
                    for j in range(2):
                        nc.scalar.dma_start(wa[:, cp, j, :],
                                            waT_v[:, 2 * cp + j, :])
                        nc.sync.dma_start(wb[:, cp, j, :],
                                          wbT_v[:, 2 * cp + j, :])
                nc.sync.dma_start(bias_t[:], bias_rep[:])
                nc.sync.dma_start(at8[:].rearrange("p c j r -> p (c j r)"),
                                  aT8[:])
                nc.sync.dma_start(bt[:], bT[:])
                nc.sync.dma_start(mall[:], maskM[:])

                def base_mms(psums, xa, xb, cp, tb, start, stop_at=None):
                    # 3 DoubleRow passes for chunk-pair cp of a 128-token
                    # block: stationary xa (wa + wb o-sweeps), then
                    # stationary xb (wa o-sweep).  stop_at o-list marks the
                    # accumulation-group end.
                    for gi, (xs, ws) in enumerate(
                            ((xa, wa), (xa, wb), (xb, wa))):
                        for o in range(N_OT):
                            nc.tensor.matmul(
                                psums[o][:],
                                xs[:, cp, :, tb:tb + 128],
                                ws[:, cp, :, o * 512:(o + 1) * 512],
                                start=(start and gi == 0),
                                stop=(stop_at is not None and gi == 2),
                                perf_mode=_DR)

                def shrink_mm(ps_s, xa_s, cp):
                    nc.tensor.matmul(ps_s[:], at8[:, cp], xa_s[:, cp],
                                     start=(cp == 0), stop=(cp == CP - 1),
                                     perf_mode=_DR)

                def shrink_fin(ps_s, s, sm):
                    # signed mask: +1 on true-adapter rows, -1 on folded
                    # majority rows, 0 for non-minority tokens.
                    mcol = (s - (N_SB - n_fix)) * SB_T
                    nc.vector.tensor_tensor(
                        sm[:], ps_s[:], mall[:, mcol:mcol + SB_T],
                        mybir.AluOpType.mult)

                def expand_mm(psums, sm, tb):
                    # bf16 rank-128 expand appended to the fp8 accumulation
                    # group (PSUM accumulate is dtype-agnostic fp32); it is
                    # the LAST matmul of each o-tile's group.
                    for o in range(N_OT):
                        nc.tensor.matmul(
                            psums[o][:], sm[:, tb:tb + 128],
                            bt[:, o * 512:(o + 1) * 512],
                            start=False, stop=True,
                            skip_group_check=True)

                def drain(ot_ap, ps_ap, col0, ncol):
                    # out = psum * (1/s) + bias
                    nc.vector.scalar_tensor_tensor(
                        ot_ap, ps_ap, float(inv_s),
                        bias_t[:, col0:col0 + ncol],
                        mybir.AluOpType.mult, mybir.AluOpType.add)

                def drain_store(ps_ap, row0, col0, ncol, ring):
                    ot = opool.tile([128, ncol], f32, tag="otq", name="otq")
                    drain(ot[:], ps_ap, col0, ncol)
                    ring.dma_start(out[row0:row0 + 128, col0:col0 + ncol],
                                   ot[:])

                sms = [None] * N_SB
                for s in range(N_SB):
                    t0 = s * SB_T
                    fix = s in fix_set
                    if xdma and s + 1 < N_SB:
                        xts[s + 1] = load_x(s + 1)
                    xa_s, xb_s = xts[s]
                    if fix and s == 0:
                        # degenerate n_fix=N_SB case: own shrink up front.
                        ps_s = pso.tile([128, 512], f32, tag="ps_o",
                                        name="ps_s")
                        for cp in range(CP):
                            shrink_mm(ps_s, xa_s, cp)
                        sms[0] = spool.tile([128, SB_T], bf16, tag="sm",
                                            name="sm0")
                        shrink_fin(ps_s, 0, sms[0])
                    if s == 0 and 1 not in fix_set:
                        # SB0: paired blocks c-major to match W delivery.
                        for bp in range(N_BLK // 2):
                            psums = [
                                pso.tile([128, 512], f32, tag="ps_o",
                                         name=f"ps_p{b}o{o}")
                                for b in range(2) for o in range(N_OT)
                            ]
                            for cp in range(CP):
                                for b in range(2):
                                    tb = (2 * bp + b) * 128
                                    base_mms(psums[b * N_OT:(b + 1) * N_OT],
                                             xa_s, xb_s, cp, tb,
                                             start=(cp == 0),
                                             stop_at=(N_OT - 1 if
                                                      cp == CP - 1 else None))
                            if store:
                                for b in range(2):
                                    ot = opool.tile([128, D_OUT], f32,
                                                    tag="ot", name="ot")
                                    tb = (2 * bp + b) * 128
                                    for o in range(N_OT):
                                        drain(ot[:, o * 512:(o + 1) * 512],
                                              psums[b * N_OT + o][:],
                                              o * 512, 512)
                                    nc.scalar.dma_start(
                                        out[t0 + tb:t0 + tb + 128, :], ot[:])
                        continue
                    for b in range(N_BLK):
                        tb = b * 128
                        last = (s == N_SB - 1 and b == N_BLK - 1)
                        if last and store:
                            # final block o-major; last o-tile in halves.
                            for o in range(N_OT):
                                ps = pso.tile([128, 512], f32, tag="ps_o",
                                              name=f"ps_l{o}")
                                for cp in range(CP):
                                    for gi, (xs, ws) in enumerate(
                                            ((xa_s, wa), (xa_s, wb),
                                             (xb_s, wa))):
                                        nc.tensor.matmul(
                                            ps[:],
                                            xs[:, cp, :, tb:tb + 128],
                                            ws[:, cp, :,
                                               o * 512:(o + 1) * 512],
                                            start=(cp == 0 and gi == 0),
                                            stop=(not fix and cp == CP - 1
                                                  and gi == 2),
                                            perf_mode=_DR)
                                if fix:
                                    nc.tensor.matmul(
                                        ps[:], sms[s][:, tb:tb + 128],
                                        bt[:, o * 512:(o + 1) * 512],
                                        start=False, stop=True,
                                        skip_group_check=True)
                                if o < N_OT - 1:
                                    drain_store(ps[:], t0 + tb, o * 512, 512,
                                                nc.scalar if o % 2
                                                else nc.sync)
                                else:
                                    for h in range(2):
                                        drain_store(
                                            ps[:, h * 256:(h + 1) * 256],
                                            t0 + tb, o * 512 + h * 256, 256,
                                            nc.scalar if h else nc.sync)
                            continue
                        # next fixup SB's shrink interleaves 1-per-chunk-pair
                        # into block 1.
                        inter_shrink = (s + 1) in fix_set and b == 1
                        if inter_shrink:
                            xa_n, _ = xts[s + 1] if xdma else xts[s]
                            ps_s = pso.tile([128, 512], f32, tag="ps_o",
                                            name="ps_s")
                        psums = [
                            pso.tile([128, 512], f32, tag="ps_o",
                                     name=f"ps_o{o}")
                            for o in range(N_OT)
                        ]
                        for cp in range(CP):
                            base_mms(psums, xa_s, xb_s, cp, tb,
                                     start=(cp == 0),
                                     stop_at=None if fix else
                                     (N_OT - 1 if cp == CP - 1 else None))
                            if inter_shrink:
                                shrink_mm(ps_s, xa_n, cp)
                        if fix:
                            expand_mm(psums, sms[s], tb)
                        if inter_shrink:
                            sms[s + 1] = spool.tile([128, SB_T], bf16,
                                                    tag="sm", name=f"sm{s+1}")
                            shrink_fin(ps_s, s + 1, sms[s + 1])
                        if not store:
                            continue
                        ot = opool.tile([128, D_OUT], f32, tag="ot", name="ot")
                        for o in range(N_OT):
                            drain(ot[:, o * 512:(o + 1) * 512], psums[o][:],
                                  o * 512, 512)
                        nc.scalar.dma_start(out[t0 + tb:t0 + tb + 128, :],
                                            ot[:])

    nc.compile()
    _CACHED[key] = nc
    return nc


def _plan_tokens(idx):
    """Sort tokens by adapter, shard contiguously, rotate minority to tail.

    Returns (perm [T_FULL] final token order, majs [N_CORES], n_fix).
    perm[c*T_CORE + j] = original token id in slot j of core c.
    """
    idx = np.asarray(idx, dtype=np.int32)
    order = np.argsort(idx, kind="stable")
    perm = np.empty_like(order)
    majs = np.empty(N_CORES, dtype=np.int64)
    max_min = 0
    for c in range(N_CORES):
        tok = order[c * T_CORE:(c + 1) * T_CORE]
        sub = idx[tok]
        majs[c] = np.bincount(sub, minlength=MAX_LORAS).argmax()
        is_min = sub != majs[c]
        max_min = max(max_min, int(is_min.sum()))
        perm[c * T_CORE:(c + 1) * T_CORE] = np.concatenate(
            [tok[~is_min], tok[is_min]])
    n_fix = max(1, -(-max_min // SB_T))
    return perm, majs, n_fix


def _retile_x(xq):
    # [T_CORE, D_IN] -> [N_SB*128, KC*SB_T] with
    # out[s*128+p, c*SB_T+t] = xq[s*SB_T+t, c*128+p]
    return np.ascontiguousarray(
        xq.reshape(N_SB, SB_T, KC, 128).transpose(0, 3, 2, 1)
        .reshape(N_SB * 128, KC * SB_T))


def _prep_inputs(x, base_weight, base_bias, lora_a, lora_b, token_lora_indices):
    bf16 = ml_dtypes.bfloat16
    f8 = ml_dtypes.float8_e4m3fn
    x = np.asarray(x, dtype=np.float32)
    w = np.asarray(base_weight, dtype=np.float32)
    bias = np.asarray(base_bias, dtype=np.float32)
    la = np.asarray(lora_a, dtype=np.float32)[:, 0]   # [L, RANK, D_IN]
    lb = np.asarray(lora_b, dtype=np.float32)[:, 0]   # [L, D_OUT, RANK]
    idx = np.asarray(token_lora_indices, dtype=np.int32)

    perm, majs, n_fix = _plan_tokens(idx)

    s = float(1.0 / w.std())         # keep wb residuals clear of denormals
    inv_s = 1.0 / s

    aT8 = np.ascontiguousarray(
        (la.reshape(128, D_IN) * s).reshape(128, KC, 128)
        .transpose(2, 1, 0).reshape(128, KC * 128)).astype(f8)       # [128, KC*128]
    bT = np.ascontiguousarray(
        lb.transpose(0, 2, 1).reshape(128, D_OUT)).astype(bf16)
    bias_rep = np.ascontiguousarray(
        np.broadcast_to(bias[None, :], (128, D_OUT))).astype(bf16)   # [128, D_OUT]

    # dual-fp8 split of x (shared across cores before the gather)
    xa_full = x.astype(f8)
    xb_full = (x - xa_full.astype(np.float32)).astype(f8)

    in_maps = []
    for c in range(N_CORES):
        tok = perm[c * T_CORE:(c + 1) * T_CORE]
        maj = int(majs[c])
        # folded per-core weight: s*(W + B_maj @ A_maj) -> [D_IN, D_OUT]
        w_c = (w + lb[maj] @ la[maj]) * s
        wa = w_c.astype(f8)
        wb = (w_c - wa.astype(np.float32)).astype(f8)
        waT = np.ascontiguousarray(wa.T)
        wbT = np.ascontiguousarray(wb.T)

        # signed fixup mask over the LAST n_fix super-blocks' token columns
        tail = tok[-n_fix * SB_T:]
        tidx = idx[tail]
        r = np.arange(128, dtype=np.int32)[:, None] // RANK          # [128,1]
        mask = ((r == tidx[None, :]).astype(np.float32)
                - (r == maj).astype(np.float32)).astype(bf16)

        in_maps.append({
            "xaP": _retile_x(xa_full[tok]),
            "xbP": _retile_x(xb_full[tok]),
            "waT": waT,
            "wbT": wbT,
            "aT8": aT8,
            "bT": bT,
            "maskM": np.ascontiguousarray(mask),
            "bias_rep": bias_rep,
        })
    return in_maps, perm, n_fix, inv_s


def kernel(x, base_weight, base_bias, lora_a, lora_b, token_lora_indices):
    in_maps, perm, n_fix, inv_s = _prep_inputs(
        x, base_weight, base_bias, lora_a, lora_b, token_lora_indices)
    nc = _build(inv_s, n_fix=n_fix)
    res = run_bass_kernel_spmd(nc, in_maps, list(range(N_CORES)))
    out = np.empty((T_FULL, D_OUT), dtype=np.float32)
    out[perm] = np.concatenate(
        [res.results[c]["out"] for c in range(N_CORES)], axis=0)
    return out


# revision 23
# speedup vs baseline: 1.4767x; 1.4767x over previous
"""Trainium2 Bass kernel for BaseLinearLayerWithLoRA (moe_routing).

out = x @ W^T + b  +  per-token LoRA:  out[t] += (x[t] @ A[l]^T) @ B[l]^T,  l = idx[t]

Sharding: tokens are stably sorted by adapter id on the host and split into
8 contiguous shards of 4096; shard c goes to core c.  After sorting, each
shard is dominated by ONE adapter (the few boundary "minority" tokens of a
neighboring adapter are rotated to the shard's tail).  Each core therefore
runs a plain dense GEMM against a per-core FOLDED weight table
W_c = W + B_maj A_maj — the LoRA contribution of the majority adapter costs
zero device work.  Only the last super-block (which holds the <=512 minority
tokens) runs the rank-128 stacked shrink/expand fixup, with a SIGNED mask
(+1 on the token's true adapter rows, -1 on the folded majority rows) so the
wrong folded contribution is subtracted and the right one added in a single
masked expand.  The host scatters per-core outputs back through the sort
permutation.

Per-core kernel design (all-bf16 operands with fp32 PSUM accumulation,
~2e-3 rms error vs the fp32 reference):
  - x is host-retiled to [super-block, partition, c-chunk, token] so each
    512-token super-block is one 2 MB line-rate DMA (8 KB contiguous per
    partition); the naive x^T layout would pay the <512 B read-mod-write tax.
  - Base GEMM: stationary = x^T chunk [128 d_in x 128 tokens], moving =
    W^T chunk [128, 512]; 4-wide o-sweep per stationary into 4 PSUM banks.
  - Fixup (last n_fix super-blocks only): the expand rides as a 17th
    contraction chunk (wt chunk 16 = stacked B table, xt chunk 16 = the
    signed-masked shrink S_m); the shrink's 16 MMs interleave 1-per-4 into
    the previous super-block's block-1 c-loop so its stationary reloads hide
    behind the base GEMM.
  - Bias is added during the PSUM->SBUF drain (DVE, host-replicated to 128
    rows); each block stores one full-row 1 MB tile.
  - Ring split: x stream + at on the SP HWDGE ring; W loads and out stores
    on the ACT ring; mask/B-table/bias on SWDGE — out stores never
    head-of-line-block the latency-critical x stream.
"""

import contextlib
import sys

for _p in ("/opt/trn_rl_repo", "/root/.axon_site/_ro/trn_rl_repo"):
    if _p not in sys.path:
        sys.path.insert(0, _p)

import numpy as np
import ml_dtypes

import concourse.bass as bass  # noqa: F401  (registers engines)
import concourse.mybir as mybir
import concourse.tile as tile
from concourse import bacc
from concourse.bass_utils import run_bass_kernel_spmd

N_CORES = 8
T_FULL, D_IN, D_OUT = 32768, 2048, 2048
MAX_LORAS, RANK = 8, 16
T_CORE = T_FULL // N_CORES          # 4096 tokens per core
SB_T = 512                          # super-block tokens
N_SB = T_CORE // SB_T               # 8 super-blocks
N_BLK = SB_T // 128                 # 4 token blocks per super-block
KC = D_IN // 128                    # 16 contraction chunks
N_OT = D_OUT // 512                 # 4 o-tiles (full width resident)

_CACHED = {}


def _build(reps=1, n_fix=1, lora=True, store=True, xdma=True, shrink1=False):
    # reps>1 wraps the whole body in a device-side For_i loop (same static
    # addresses each iteration) — used only by the timing harness to amortize
    # launch overhead; the graded kernel path uses reps=1.  lora/store/xdma
    # are ablation switches for HW bottleneck attribution (timing harness
    # only — they break correctness).  n_fix = number of trailing
    # super-blocks that run the minority-token fixup.
    key = ("nc", reps, n_fix, lora, store, xdma, shrink1)
    if key in _CACHED:
        return _CACHED[key]
    fix_set = set(range(N_SB - n_fix, N_SB)) if lora else set()

    f32 = mybir.dt.float32
    bf16 = mybir.dt.bfloat16

    nc = bacc.Bacc("TRN2", target_bir_lowering=False, debug=False)

    # xP[s, p, c*SB_T + t] = x[s*SB_T + t, c*128 + p]: one contiguous 8 KB run
    # per (super-block, partition) so each super-block is a single 1 MB DMA at
    # line rate (256 B runs of the naive x^T layout pay the <512 B RMW tax).
    xP = nc.dram_tensor("xP", [N_SB * 128, KC * SB_T], bf16, kind="ExternalInput")
    wT = nc.dram_tensor("wT", [D_IN, D_OUT], bf16, kind="ExternalInput")
    aT = nc.dram_tensor("aT", [D_IN, 128], bf16, kind="ExternalInput")
    bT = nc.dram_tensor("bT", [128, D_OUT], bf16, kind="ExternalInput")
    maskM = nc.dram_tensor("maskM", [128, max(n_fix, 1) * SB_T], bf16,
                           kind="ExternalInput")
    bias_rep = nc.dram_tensor("bias_rep", [128, D_OUT], bf16, kind="ExternalInput")
    out = nc.dram_tensor("out", [T_CORE, D_OUT], f32, kind="ExternalOutput")

    xP_v = xP.rearrange("(s p) q -> p s q", p=128)      # [128, N_SB, KC*SB_T]
    wT_v = wT.rearrange("(c p) o -> p c o", p=128)      # [128, 16, 2048]
    aT_v = aT.rearrange("(c p) r -> p c r", p=128)      # [128, 16, 128]

    with tile.TileContext(nc) as tc:
        with (
            tc.tile_pool(name="const", bufs=1) as const,
            tc.tile_pool(name="wpool", bufs=1) as wpool,
            tc.tile_pool(name="xpool", bufs=3) as xpool,
            tc.tile_pool(name="opool", bufs=3) as opool,
            tc.tile_pool(name="pso", bufs=8, space="PSUM") as pso,
        ):
            at = const.tile([128, KC, 128], bf16)
            bias_t = const.tile([128, D_OUT], bf16)
            mall = const.tile([128, max(n_fix, 1) * SB_T], bf16)
            # chunk KC of wt holds the stacked LoRA B table: the fixup expand
            # is just a 17th contraction chunk of the base GEMM.
            wt = wpool.tile([128, KC + 1, D_OUT], bf16)

            rep_cm = tc.For_i(0, reps) if reps > 1 else contextlib.nullcontext()
            with rep_cm:
                def load_x(s, splits=(KC,), eng=None):
                    # chunk KC is filled by shrink()'s DVE mask-multiply.
                    # multiple split groups let the first block's matmuls
                    # start after the first small group lands.
                    eng = eng or nc.sync
                    xt = xpool.tile([128, KC + 1, SB_T], bf16, tag="xt",
                                    name=f"xt{s}")
                    g0 = 0
                    for gs in splits:
                        eng.dma_start(
                            xt[:, g0:g0 + gs, :]
                            .rearrange("p c t -> p (c t)"),
                            xP_v[:, s, g0 * SB_T:(g0 + gs) * SB_T])
                        g0 += gs
                    assert g0 == KC
                    return xt

                xts = [None] * N_SB
                # Prologue: the DMA bus is the serial resource; order the
                # early traffic so W's 16 chunks (needed by SB0's paired
                # c-major sweep) finish before the bus serves anything not
                # needed until later.  SB0's x rides SWDGE in growing groups
                # (first 2-chunk group -> first matmul at ~3.5 us); W chunks
                # alternate between the SP and ACT HWDGE rings; the small
                # tables (at/mask/B/bias, needed from SB6 at the earliest)
                # queue on SP BEHIND the W odds.  Steady state is unaffected
                # (W resident; SP carries x, ACT carries out stores).
                xts[0] = load_x(0, splits=(2, 2, 4, 8), eng=nc.gpsimd)
                # W chunk 0 split across BOTH rings: its two halves win the
                # bus first, so block 0's first matmul starts ~1.4 us sooner.
                nc.sync.dma_start(wt[:, 0, :1024], wT_v[:, 0, :1024])
                nc.scalar.dma_start(wt[:, 0, 1024:], wT_v[:, 0, 1024:])
                for c in range(1, KC):
                    eng = nc.sync if c % 2 else nc.scalar
                    eng.dma_start(wt[:, c, :], wT_v[:, c, :])
                nc.sync.dma_start(bias_t[:], bias_rep[:])
                nc.sync.dma_start(at[:], aT_v[:])
                nc.sync.dma_start(wt[:, KC, :], bT[:])
                nc.sync.dma_start(mall[:], maskM[:])

                n_shr = 1 if shrink1 else KC

                def shrink_mm(ps_s, s, xt_s, c):
                    nc.tensor.matmul(ps_s[:, :SB_T], at[:, c, :],
                                     xt_s[:, c, :],
                                     start=(c == 0), stop=(c == n_shr - 1))

                def shrink_fin(ps_s, s, xt_s):
                    # signed mask: +1 on true-adapter rows, -1 on folded
                    # majority rows, 0 for non-minority tokens; result lands
                    # in xt chunk KC = the expand's lhsT.
                    mcol = (s - (N_SB - n_fix)) * SB_T
                    nc.vector.tensor_tensor(
                        xt_s[:, KC, :], ps_s[:, :SB_T],
                        mall[:, mcol:mcol + SB_T],
                        mybir.AluOpType.mult)

                def shrink(s, xt_s):
                    # standalone masked fixup shrink (prologue only)
                    ps_s = pso.tile([128, 512], f32, tag="ps_o", name="ps_s")
                    for c in range(n_shr):
                        shrink_mm(ps_s, s, xt_s, c)
                    shrink_fin(ps_s, s, xt_s)
                def drain_store(ps, row0, col0, ncol, ring):
                    ot = opool.tile([128, ncol], f32, tag="otq", name="otq")
                    nc.vector.tensor_tensor(
                        ot[:], ps,
                        bias_t[:, col0:col0 + ncol], mybir.AluOpType.add)
                    ring.dma_start(out[row0:row0 + 128, col0:col0 + ncol],
                                   ot[:])

                for s in range(N_SB):
                    t0 = s * SB_T
                    kc_eff = KC + 1 if s in fix_set else KC
                    if xdma and s + 1 < N_SB:
                        xts[s + 1] = load_x(s + 1)
                    xt = xts[s]
                    if s in fix_set and s == 0:
                        # degenerate n_fix=N_SB case: SB0's own shrink runs
                        # ahead of its block loop.
                        shrink(0, xt)
                    if s == 0 and 1 not in fix_set:
                        # Prologue: W chunks arrive on the (serial) DMA bus
                        # slower than one block consumes them (1.46 us vs
                        # 0.85 us per chunk), so SB0 runs its blocks in PAIRS
                        # c-major — two blocks share each arriving W chunk,
                        # matching consumption to delivery.  8 PSUM banks =
                        # 2 blocks x 4 o-tiles.
                        for bp in range(N_BLK // 2):
                            psums = [
                                pso.tile([128, 512], f32, tag="ps_o",
                                         name=f"ps_p{b}o{o}")
                                for b in range(2) for o in range(N_OT)
                            ]
                            for c in range(kc_eff):
                                for b in range(2):
                                    tb = (2 * bp + b) * 128
                                    for o in range(N_OT):
                                        nc.tensor.matmul(
                                            psums[b * N_OT + o][:],
                                            xt[:, c, tb:tb + 128],
                                            wt[:, c, o * 512:(o + 1) * 512],
                                            start=(c == 0),
                                            stop=(c == kc_eff - 1))
                            if store:
                                for b in range(2):
                                    ot = opool.tile([128, D_OUT], f32,
                                                    tag="ot", name="ot")
                                    tb = (2 * bp + b) * 128
                                    for o in range(N_OT):
                                        nc.vector.tensor_tensor(
                                            ot[:, o * 512:(o + 1) * 512],
                                            psums[b * N_OT + o][:],
                                            bias_t[:, o * 512:(o + 1) * 512],
                                            mybir.AluOpType.add)
                                    nc.scalar.dma_start(
                                        out[t0 + tb:t0 + tb + 128, :], ot[:])
                        continue
                    for b in range(N_BLK):
                        tb = b * 128
                        last = (s == N_SB - 1 and b == N_BLK - 1)
                        if last and store:
                            # final block runs o-MAJOR: each o-tile's
                            # accumulation finishes 17 MMs before the next,
                            # so its drain+store hides behind the next
                            # phase's matmuls; the serial tail after the very
                            # last matmul is one drain + one 0.25 MB store
                            # (vs 4 drains + 4 bus-serial stores).
                            for o in range(N_OT):
                                ps = pso.tile([128, 512], f32, tag="ps_o",
                                              name=f"ps_l{o}")
                                for c in range(kc_eff):
                                    nc.tensor.matmul(
                                        ps[:],
                                        xt[:, c, tb:tb + 128],
                                        wt[:, c, o * 512:(o + 1) * 512],
                                        start=(c == 0),
                                        stop=(c == kc_eff - 1))
                                if o < N_OT - 1:
                                    drain_store(ps[:], t0 + tb, o * 512, 512,
                                                nc.scalar if o % 2 else nc.sync)
                                else:
                                    # very last o-tile: halve the drain+store
                                    # so the serial tail is one 256-col drain
                                    # + one 0.125 MB store.
                                    for h in range(2):
                                        drain_store(
                                            ps[:, h * 256:(h + 1) * 256],
                                            t0 + tb, o * 512 + h * 256, 256,
                                            nc.scalar if h else nc.sync)
                            continue
                        # the next super-block's fixup shrink MMs interleave
                        # into block 1's c-loop (1 shrink MM per 4 base MMs):
                        # stationary reloads hide behind the base group and
                        # consecutive shrink MMs never hit the same PSUM bank
                        # back-to-back — a 16-MM same-bank burst measures
                        # ~2x its roofline cost.
                        inter_shrink = (s + 1) in fix_set and b == 1
                        if inter_shrink:
                            xt_n = xts[s + 1] if xdma else xt
                            ps_s = pso.tile([128, 512], f32, tag="ps_o",
                                            name="ps_s")
                        psums = [
                            pso.tile([128, 512], f32, tag="ps_o",
                                     name=f"ps_o{o}")
                            for o in range(N_OT)
                        ]
                        for c in range(kc_eff):
                            for o in range(N_OT):
                                nc.tensor.matmul(
                                    psums[o][:],
                                    xt[:, c, tb:tb + 128],
                                    wt[:, c, o * 512:(o + 1) * 512],
                                    start=(c == 0),
                                    stop=(c == kc_eff - 1))
                            if inter_shrink and c < n_shr:
                                shrink_mm(ps_s, s + 1, xt_n, c)
                        if inter_shrink:
                            shrink_fin(ps_s, s + 1, xt_n)
                        if not store:
                            continue
                        ot = opool.tile([128, D_OUT], f32, tag="ot", name="ot")
                        for o in range(N_OT):
                            nc.vector.tensor_tensor(
                                ot[:, o * 512:(o + 1) * 512], psums[o][:],
                                bias_t[:, o * 512:(o + 1) * 512],
                                mybir.AluOpType.add)
                        # one full-row 1 MB store per block (8 KB/partition
                        # contiguous, line-rate) on the ACT ring, which is
                        # idle once W is loaded — never blocks the x
                        # stream.
                        nc.scalar.dma_start(out[t0 + tb:t0 + tb + 128, :],
                                            ot[:])

    nc.compile()
    _CACHED[key] = nc
    return nc


def _plan_tokens(idx):
    """Sort tokens by adapter, shard contiguously, rotate minority to tail.

    Returns (perm [T_FULL] final token order, majs [N_CORES], n_fix).
    perm[c*T_CORE + j] = original token id in slot j of core c.
    """
    idx = np.asarray(idx, dtype=np.int32)
    order = np.argsort(idx, kind="stable")
    perm = np.empty_like(order)
    majs = np.empty(N_CORES, dtype=np.int64)
    max_min = 0
    for c in range(N_CORES):
        tok = order[c * T_CORE:(c + 1) * T_CORE]
        sub = idx[tok]
        majs[c] = np.bincount(sub, minlength=MAX_LORAS).argmax()
        is_min = sub != majs[c]
        max_min = max(max_min, int(is_min.sum()))
        # stable partition: majority tokens first, minority at the tail
        perm[c * T_CORE:(c + 1) * T_CORE] = np.concatenate(
            [tok[~is_min], tok[is_min]])
    n_fix = max(1, -(-max_min // SB_T))
    return perm, majs, n_fix


def _prep_inputs(x, base_weight, base_bias, lora_a, lora_b, token_lora_indices):
    bf16 = ml_dtypes.bfloat16
    x = np.asarray(x, dtype=np.float32)
    w = np.asarray(base_weight, dtype=np.float32)
    bias = np.asarray(base_bias, dtype=np.float32)
    la = np.asarray(lora_a, dtype=np.float32)[:, 0]   # [L, RANK, D_IN]
    lb = np.asarray(lora_b, dtype=np.float32)[:, 0]   # [L, D_OUT, RANK]
    idx = np.asarray(token_lora_indices, dtype=np.int32)

    perm, majs, n_fix = _plan_tokens(idx)

    aT = np.ascontiguousarray(la.reshape(128, D_IN).T).astype(bf16)  # [D_IN, 128]
    bT = np.ascontiguousarray(
        lb.transpose(0, 2, 1).reshape(128, D_OUT)).astype(bf16)
    bias_rep = np.ascontiguousarray(
        np.broadcast_to(bias[None, :], (128, D_OUT))).astype(bf16)   # [128, D_OUT]

    in_maps = []
    for c in range(N_CORES):
        tok = perm[c * T_CORE:(c + 1) * T_CORE]
        maj = int(majs[c])
        # folded per-core weight: W + B_maj @ A_maj, transposed to [D_IN, D_OUT]
        w_c = w + lb[maj] @ la[maj]
        wT = np.ascontiguousarray(w_c.T).astype(bf16)

        # signed fixup mask over the LAST n_fix super-blocks' token columns:
        # +1 on the token's true adapter rows, -1 on the folded majority rows,
        # 0 for majority tokens.
        tail = tok[-n_fix * SB_T:]
        tidx = idx[tail]
        r = np.arange(128, dtype=np.int32)[:, None] // RANK          # [128,1]
        # (r==tidx) - (r==maj) cancels to all-zero for majority tokens.
        mask = ((r == tidx[None, :]).astype(np.float32)
                - (r == maj).astype(np.float32)).astype(bf16)

        # xP[s, p, c, t] = x[tok[s*SB_T + t], c*128 + p]: one contiguous 8 KB
        # run per (super-block, partition) -> line-rate DMA.
        xP = (x[tok].reshape(N_SB, SB_T, KC, 128)
              .transpose(0, 3, 2, 1)
              .reshape(N_SB * 128, KC * SB_T).astype(bf16))
        in_maps.append({
            "xP": np.ascontiguousarray(xP),
            "wT": wT,
            "aT": aT,
            "bT": bT,
            "maskM": np.ascontiguousarray(mask),
            "bias_rep": bias_rep,
        })
    return in_maps, perm, n_fix


def kernel(x, base_weight, base_bias, lora_a, lora_b, token_lora_indices):
    in_maps, perm, n_fix = _prep_inputs(x, base_weight, base_bias, lora_a,
                                        lora_b, token_lora_indices)
    nc = _build(n_fix=n_fix)
    res = run_bass_kernel_spmd(nc, in_maps, list(range(N_CORES)))
    out = np.empty((T_FULL, D_OUT), dtype=np.float32)
    out[perm] = np.concatenate(
        [res.results[c]["out"] for c in range(N_CORES)], axis=0)
    return out


# revision 28
# speedup vs baseline: 1.5159x; 1.0265x over previous
"""Trainium2 Bass kernel for BaseLinearLayerWithLoRA (moe_routing).

out = x @ W^T + b  +  per-token LoRA:  out[t] += (x[t] @ A[l]^T) @ B[l]^T,  l = idx[t]

Sharding: tokens are stably sorted by adapter id on the host and split into
8 contiguous shards of 4096; shard c goes to core c.  After sorting, each
shard is dominated by ONE adapter (the few boundary "minority" tokens of a
neighboring adapter are rotated to the shard's tail).  Each core therefore
runs a plain dense GEMM against a per-core FOLDED weight table
W_c = W + B_maj A_maj — the LoRA contribution of the majority adapter costs
zero device work.  Only the last super-block (which holds the <=512 minority
tokens) runs the rank-128 stacked shrink/expand fixup, with a SIGNED mask
(+1 on the token's true adapter rows, -1 on the folded majority rows) so the
wrong folded contribution is subtracted and the right one added in a single
masked expand.  The host scatters per-core outputs back through the sort
permutation.

Per-core kernel design (all-bf16 operands with fp32 PSUM accumulation,
~2e-3 rms error vs the fp32 reference):
  - x is host-retiled to [super-block, partition, c-chunk, token] so each
    512-token super-block is one 2 MB line-rate DMA (8 KB contiguous per
    partition); the naive x^T layout would pay the <512 B read-mod-write tax.
  - Base GEMM: stationary = x^T chunk [128 d_in x 128 tokens], moving =
    W^T chunk [128, 512]; 4-wide o-sweep per stationary into 4 PSUM banks.
  - Fixup (last n_fix super-blocks only): the expand rides as a 17th
    contraction chunk (wt chunk 16 = stacked B table, xt chunk 16 = the
    signed-masked shrink S_m); the shrink's 16 MMs interleave 1-per-4 into
    the previous super-block's block-1 c-loop so its stationary reloads hide
    behind the base GEMM.
  - Bias is added during the PSUM->SBUF drain (DVE, host-replicated to 128
    rows); each block stores one full-row 1 MB tile.
  - Ring split: x stream + at on the SP HWDGE ring; W loads and out stores
    on the ACT ring; mask/B-table/bias on SWDGE — out stores never
    head-of-line-block the latency-critical x stream.
"""

import contextlib
import sys

for _p in ("/opt/trn_rl_repo", "/root/.axon_site/_ro/trn_rl_repo"):
    if _p not in sys.path:
        sys.path.insert(0, _p)

import numpy as np
import ml_dtypes

import concourse.bass as bass  # noqa: F401  (registers engines)
import concourse.mybir as mybir
import concourse.tile as tile
from concourse import bacc
from concourse.bass_utils import run_bass_kernel_spmd

N_CORES = 8
T_FULL, D_IN, D_OUT = 32768, 2048, 2048
MAX_LORAS, RANK = 8, 16
T_CORE = T_FULL // N_CORES          # 4096 tokens per core
SB_T = 512                          # super-block tokens
N_SB = T_CORE // SB_T               # 8 super-blocks
N_BLK = SB_T // 128                 # 4 token blocks per super-block
KC = D_IN // 128                    # 16 contraction chunks
N_OT = D_OUT // 512                 # 4 o-tiles (full width resident)

_CACHED = {}


def _build(reps=1, n_fix=1, lora=True, store=True, xdma=True, shrink1=False):
    # reps>1 wraps the whole body in a device-side For_i loop (same static
    # addresses each iteration) — used only by the timing harness to amortize
    # launch overhead; the graded kernel path uses reps=1.  lora/store/xdma
    # are ablation switches for HW bottleneck attribution (timing harness
    # only — they break correctness).  n_fix = number of trailing
    # super-blocks that run the minority-token fixup.
    key = ("nc", reps, n_fix, lora, store, xdma, shrink1)
    if key in _CACHED:
        return _CACHED[key]
    fix_set = set(range(N_SB - n_fix, N_SB)) if lora else set()

    f32 = mybir.dt.float32
    bf16 = mybir.dt.bfloat16

    nc = bacc.Bacc("TRN2", target_bir_lowering=False, debug=False)

    # xP[s, p, c*SB_T + t] = x[s*SB_T + t, c*128 + p]: one contiguous 8 KB run
    # per (super-block, partition) so each super-block is a single 1 MB DMA at
    # line rate (256 B runs of the naive x^T layout pay the <512 B RMW tax).
    xP = nc.dram_tensor("xP", [N_SB * 128, KC * SB_T], bf16, kind="ExternalInput")
    wT = nc.dram_tensor("wT", [D_IN, D_OUT], bf16, kind="ExternalInput")
    aT = nc.dram_tensor("aT", [D_IN, 128], bf16, kind="ExternalInput")
    bT = nc.dram_tensor("bT", [128, D_OUT], bf16, kind="ExternalInput")
    maskM = nc.dram_tensor("maskM", [128, max(n_fix, 1) * SB_T], bf16,
                           kind="ExternalInput")
    bias_rep = nc.dram_tensor("bias_rep", [128, D_OUT], bf16, kind="ExternalInput")
    # out is stored bf16 (halves store traffic; host upcasts to f32 during
    # the unshard gather — adds ~2e-3 rounding, still well under the gate).
    out = nc.dram_tensor("out", [T_CORE, D_OUT], bf16, kind="ExternalOutput")

    xP_v = xP.rearrange("(s p) q -> p s q", p=128)      # [128, N_SB, KC*SB_T]
    wT_v = wT.rearrange("(c p) o -> p c o", p=128)      # [128, 16, 2048]
    aT_v = aT.rearrange("(c p) r -> p c r", p=128)      # [128, 16, 128]

    with tile.TileContext(nc) as tc:
        with (
            tc.tile_pool(name="const", bufs=1) as const,
            tc.tile_pool(name="wpool", bufs=1) as wpool,
            tc.tile_pool(name="xpool", bufs=3) as xpool,
            tc.tile_pool(name="opool", bufs=3) as opool,
            tc.tile_pool(name="pso", bufs=8, space="PSUM") as pso,
        ):
            at = const.tile([128, KC, 128], bf16)
            bias_t = const.tile([128, D_OUT], bf16)
            mall = const.tile([128, max(n_fix, 1) * SB_T], bf16)
            # scratch operands for the HAM warmup burst
            wup_a = const.tile([128, 128], bf16)
            wup_b = const.tile([128, 512], bf16)
            # chunk KC of wt holds the stacked LoRA B table: the fixup expand
            # is just a 17th contraction chunk of the base GEMM.
            wt = wpool.tile([128, KC + 1, D_OUT], bf16)

            rep_cm = tc.For_i(0, reps) if reps > 1 else contextlib.nullcontext()
            with rep_cm:
                def load_x(s, splits=(KC,), eng=None):
                    # chunk KC is filled by shrink()'s DVE mask-multiply.
                    # multiple split groups let the first block's matmuls
                    # start after the first small group lands.
                    eng = eng or nc.sync
                    xt = xpool.tile([128, KC + 1, SB_T], bf16, tag="xt",
                                    name=f"xt{s}")
                    g0 = 0
                    for gs in splits:
                        eng.dma_start(
                            xt[:, g0:g0 + gs, :]
                            .rearrange("p c t -> p (c t)"),
                            xP_v[:, s, g0 * SB_T:(g0 + gs) * SB_T])
                        g0 += gs
                    assert g0 == KC
                    return xt

                # HAM warmup: the PE clock-gate runs at 1.2 GHz until ~3.4 us
                # of sustained matmul activity.  The DMA prologue leaves the
                # PE idle ~4 us anyway, so burn it on dummy matmuls over
                # zeroed scratch — real matmuls then start at 2.4 GHz.  Also
                # keeps HAM warm across the For_i timing reps.
                nc.vector.memzero(wup_a[:])
                nc.vector.memzero(wup_b[:])
                ps_w = [pso.tile([128, 512], f32, tag="ps_o", name=f"ps_w{i}")
                        for i in range(2)]
                for i in range(12):
                    nc.tensor.matmul(ps_w[i % 2][:], wup_a[:], wup_b[:],
                                     start=(i < 2), stop=(i >= 10))

                xts = [None] * N_SB
                # Prologue: the DMA bus is the serial resource; order the
                # early traffic so W's 16 chunks (needed by SB0's paired
                # c-major sweep) finish before the bus serves anything not
                # needed until later.  SB0's x rides SWDGE in growing groups
                # (first 2-chunk group -> first matmul at ~3.5 us); W chunks
                # alternate between the SP and ACT HWDGE rings; the small
                # tables (at/mask/B/bias, needed from SB6 at the earliest)
                # queue on SP BEHIND the W odds.  Steady state is unaffected
                # (W resident; SP carries x, ACT carries out stores).
                xts[0] = load_x(0, splits=(2, 2, 4, 8), eng=nc.gpsimd)
                # W chunk 0 split across BOTH rings: its two halves win the
                # bus first, so block 0's first matmul starts ~1.4 us sooner.
                nc.sync.dma_start(wt[:, 0, :1024], wT_v[:, 0, :1024])
                nc.scalar.dma_start(wt[:, 0, 1024:], wT_v[:, 0, 1024:])
                for c in range(1, KC):
                    eng = nc.sync if c % 2 else nc.scalar
                    eng.dma_start(wt[:, c, :], wT_v[:, c, :])
                nc.sync.dma_start(bias_t[:], bias_rep[:])
                nc.sync.dma_start(at[:], aT_v[:])
                nc.sync.dma_start(wt[:, KC, :], bT[:])
                nc.sync.dma_start(mall[:], maskM[:])

                n_shr = 1 if shrink1 else KC

                def shrink_mm(ps_s, s, xt_s, c):
                    nc.tensor.matmul(ps_s[:, :SB_T], at[:, c, :],
                                     xt_s[:, c, :],
                                     start=(c == 0), stop=(c == n_shr - 1))

                def shrink_fin(ps_s, s, xt_s):
                    # signed mask: +1 on true-adapter rows, -1 on folded
                    # majority rows, 0 for non-minority tokens; result lands
                    # in xt chunk KC = the expand's lhsT.
                    mcol = (s - (N_SB - n_fix)) * SB_T
                    nc.vector.tensor_tensor(
                        xt_s[:, KC, :], ps_s[:, :SB_T],
                        mall[:, mcol:mcol + SB_T],
                        mybir.AluOpType.mult)

                def shrink(s, xt_s):
                    # standalone masked fixup shrink (prologue only)
                    ps_s = pso.tile([128, 512], f32, tag="ps_o", name="ps_s")
                    for c in range(n_shr):
                        shrink_mm(ps_s, s, xt_s, c)
                    shrink_fin(ps_s, s, xt_s)
                def drain_store(ps, row0, col0, ncol, ring):
                    ot = opool.tile([128, ncol], bf16, tag="otq", name="otq")
                    nc.vector.tensor_tensor(
                        ot[:], ps,
                        bias_t[:, col0:col0 + ncol], mybir.AluOpType.add)
                    ring.dma_start(out[row0:row0 + 128, col0:col0 + ncol],
                                   ot[:])

                for s in range(N_SB):
                    t0 = s * SB_T
                    kc_eff = KC + 1 if s in fix_set else KC
                    if xdma and s + 1 < N_SB:
                        xts[s + 1] = load_x(s + 1)
                    xt = xts[s]
                    if s in fix_set and s == 0:
                        # degenerate n_fix=N_SB case: SB0's own shrink runs
                        # ahead of its block loop.
                        shrink(0, xt)
                    if s == 0 and 1 not in fix_set:
                        # Prologue: W chunks arrive on the (serial) DMA bus
                        # slower than one block consumes them (1.46 us vs
                        # 0.85 us per chunk), so SB0 runs its blocks in PAIRS
                        # c-major — two blocks share each arriving W chunk,
                        # matching consumption to delivery.  8 PSUM banks =
                        # 2 blocks x 4 o-tiles.
                        for bp in range(N_BLK // 2):
                            psums = [
                                pso.tile([128, 512], f32, tag="ps_o",
                                         name=f"ps_p{b}o{o}")
                                for b in range(2) for o in range(N_OT)
                            ]
                            for c in range(kc_eff):
                                for b in range(2):
                                    tb = (2 * bp + b) * 128
                                    for o in range(N_OT):
                                        nc.tensor.matmul(
                                            psums[b * N_OT + o][:],
                                            xt[:, c, tb:tb + 128],
                                            wt[:, c, o * 512:(o + 1) * 512],
                                            start=(c == 0),
                                            stop=(c == kc_eff - 1))
                            if store:
                                for b in range(2):
                                    ot = opool.tile([128, D_OUT], bf16,
                                                    tag="ot", name="ot")
                                    tb = (2 * bp + b) * 128
                                    for o in range(N_OT):
                                        nc.vector.tensor_tensor(
                                            ot[:, o * 512:(o + 1) * 512],
                                            psums[b * N_OT + o][:],
                                            bias_t[:, o * 512:(o + 1) * 512],
                                            mybir.AluOpType.add)
                                    nc.scalar.dma_start(
                                        out[t0 + tb:t0 + tb + 128, :], ot[:])
                        continue
                    for b in range(N_BLK):
                        tb = b * 128
                        last = (s == N_SB - 1 and b == N_BLK - 1)
                        if last and store:
                            # final block runs o-MAJOR: each o-tile's
                            # accumulation finishes 17 MMs before the next,
                            # so its drain+store hides behind the next
                            # phase's matmuls; the serial tail after the very
                            # last matmul is one drain + one 0.25 MB store
                            # (vs 4 drains + 4 bus-serial stores).
                            for o in range(N_OT):
                                ps = pso.tile([128, 512], f32, tag="ps_o",
                                              name=f"ps_l{o}")
                                for c in range(kc_eff):
                                    nc.tensor.matmul(
                                        ps[:],
                                        xt[:, c, tb:tb + 128],
                                        wt[:, c, o * 512:(o + 1) * 512],
                                        start=(c == 0),
                                        stop=(c == kc_eff - 1))
                                if o < N_OT - 1:
                                    drain_store(ps[:], t0 + tb, o * 512, 512,
                                                nc.scalar if o % 2 else nc.sync)
                                else:
                                    # very last o-tile: halve the drain+store
                                    # so the serial tail is one 256-col drain
                                    # + one 0.125 MB store.
                                    for h in range(2):
                                        drain_store(
                                            ps[:, h * 256:(h + 1) * 256],
                                            t0 + tb, o * 512 + h * 256, 256,
                                            nc.scalar if h else nc.sync)
                            continue
                        # the next super-block's fixup shrink MMs interleave
                        # into block 1's c-loop (1 shrink MM per 4 base MMs):
                        # stationary reloads hide behind the base group and
                        # consecutive shrink MMs never hit the same PSUM bank
                        # back-to-back — a 16-MM same-bank burst measures
                        # ~2x its roofline cost.
                        inter_shrink = (s + 1) in fix_set and b == 1
                        if inter_shrink:
                            xt_n = xts[s + 1] if xdma else xt
                            ps_s = pso.tile([128, 512], f32, tag="ps_o",
                                            name="ps_s")
                        psums = [
                            pso.tile([128, 512], f32, tag="ps_o",
                                     name=f"ps_o{o}")
                            for o in range(N_OT)
                        ]
                        for c in range(kc_eff):
                            for o in range(N_OT):
                                nc.tensor.matmul(
                                    psums[o][:],
                                    xt[:, c, tb:tb + 128],
                                    wt[:, c, o * 512:(o + 1) * 512],
                                    start=(c == 0),
                                    stop=(c == kc_eff - 1))
                            if inter_shrink and c < n_shr:
                                shrink_mm(ps_s, s + 1, xt_n, c)
                        if inter_shrink:
                            shrink_fin(ps_s, s + 1, xt_n)
                        if not store:
                            continue
                        ot = opool.tile([128, D_OUT], bf16, tag="ot", name="ot")
                        for o in range(N_OT):
                            nc.vector.tensor_tensor(
                                ot[:, o * 512:(o + 1) * 512], psums[o][:],
                                bias_t[:, o * 512:(o + 1) * 512],
                                mybir.AluOpType.add)
                        # one full-row 1 MB store per block (8 KB/partition
                        # contiguous, line-rate) on the ACT ring, which is
                        # idle once W is loaded — never blocks the x
                        # stream.
                        nc.scalar.dma_start(out[t0 + tb:t0 + tb + 128, :],
                                            ot[:])

    nc.compile()
    _CACHED[key] = nc
    return nc


def _plan_tokens(idx):
    """Sort tokens by adapter, shard contiguously, rotate minority to tail.

    Returns (perm [T_FULL] final token order, majs [N_CORES], n_fix).
    perm[c*T_CORE + j] = original token id in slot j of core c.
    """
    idx = np.asarray(idx, dtype=np.int32)
    order = np.argsort(idx, kind="stable")
    perm = np.empty_like(order)
    majs = np.empty(N_CORES, dtype=np.int64)
    max_min = 0
    for c in range(N_CORES):
        tok = order[c * T_CORE:(c + 1) * T_CORE]
        sub = idx[tok]
        majs[c] = np.bincount(sub, minlength=MAX_LORAS).argmax()
        is_min = sub != majs[c]
        max_min = max(max_min, int(is_min.sum()))
        # stable partition: majority tokens first, minority at the tail
        perm[c * T_CORE:(c + 1) * T_CORE] = np.concatenate(
            [tok[~is_min], tok[is_min]])
    n_fix = max(1, -(-max_min // SB_T))
    return perm, majs, n_fix


def _prep_inputs(x, base_weight, base_bias, lora_a, lora_b, token_lora_indices):
    bf16 = ml_dtypes.bfloat16
    x = np.asarray(x, dtype=np.float32)
    w = np.asarray(base_weight, dtype=np.float32)
    bias = np.asarray(base_bias, dtype=np.float32)
    la = np.asarray(lora_a, dtype=np.float32)[:, 0]   # [L, RANK, D_IN]
    lb = np.asarray(lora_b, dtype=np.float32)[:, 0]   # [L, D_OUT, RANK]
    idx = np.asarray(token_lora_indices, dtype=np.int32)

    perm, majs, n_fix = _plan_tokens(idx)

    aT = np.ascontiguousarray(la.reshape(128, D_IN).T).astype(bf16)  # [D_IN, 128]
    bT = np.ascontiguousarray(
        lb.transpose(0, 2, 1).reshape(128, D_OUT)).astype(bf16)
    bias_rep = np.ascontiguousarray(
        np.broadcast_to(bias[None, :], (128, D_OUT))).astype(bf16)   # [128, D_OUT]

    in_maps = []
    for c in range(N_CORES):
        tok = perm[c * T_CORE:(c + 1) * T_CORE]
        maj = int(majs[c])
        # folded per-core weight: W + B_maj @ A_maj, transposed to [D_IN, D_OUT]
        w_c = w + lb[maj] @ la[maj]
        wT = np.ascontiguousarray(w_c.T).astype(bf16)

        # signed fixup mask over the LAST n_fix super-blocks' token columns:
        # +1 on the token's true adapter rows, -1 on the folded majority rows,
        # 0 for majority tokens.
        tail = tok[-n_fix * SB_T:]
        tidx = idx[tail]
        r = np.arange(128, dtype=np.int32)[:, None] // RANK          # [128,1]
        # (r==tidx) - (r==maj) cancels to all-zero for majority tokens.
        mask = ((r == tidx[None, :]).astype(np.float32)
                - (r == maj).astype(np.float32)).astype(bf16)

        # xP[s, p, c, t] = x[tok[s*SB_T + t], c*128 + p]: one contiguous 8 KB
        # run per (super-block, partition) -> line-rate DMA.
        xP = (x[tok].reshape(N_SB, SB_T, KC, 128)
              .transpose(0, 3, 2, 1)
              .reshape(N_SB * 128, KC * SB_T).astype(bf16))
        in_maps.append({
            "xP": np.ascontiguousarray(xP),
            "wT": wT,
            "aT": aT,
            "bT": bT,
            "maskM": np.ascontiguousarray(mask),
            "bias_rep": bias_rep,
        })
    return in_maps, perm, n_fix


def kernel(x, base_weight, base_bias, lora_a, lora_b, token_lora_indices):
    in_maps, perm, n_fix = _prep_inputs(x, base_weight, base_bias, lora_a,
                                        lora_b, token_lora_indices)
    nc = _build(n_fix=n_fix)
    res = run_bass_kernel_spmd(nc, in_maps, list(range(N_CORES)))
    out = np.empty((T_FULL, D_OUT), dtype=np.float32)
    out[perm] = np.concatenate(
        [res.results[c]["out"] for c in range(N_CORES)], axis=0)
    return out
